# revision 21
# baseline (speedup 1.0000x reference)
"""Trainium2 Bass kernel for nn_AttentionBlock (GroupNorm + single-head
attention over N=HW + 1x1 convs + residual).

Sharding: data-parallel over batch. B=16 across 8 cores -> 2 batch elements
per core, no collectives.

Per-core pipeline (per batch element, layouts chosen so no PE transposes are
ever needed):
  x        [C=512(part,4x128), N=1024(free)]  fp32
  GroupNorm: bn_stats/bn_aggr per channel, cross-partition group reduce via a
             tiny fp32 matmul with a 0/1 group-selection matrix, broadcast
             back via its transpose; h = x*a + b  (a,b per-channel) -> f32r
  qkv:      q,k as [C, N] = W^T-tiles.T @ h;  v directly transposed as
            vT [N, C] = h-tiles.T @ WvT  (free!)
  scores:   S^T [m, n] = k.T @ q  (lhsT=k tile, rhs=q)  -> PSUM
  softmax:  exp on ACT straight out of PSUM (scale folded into activation),
            no max-subtraction (scores are O(5), fp32 exp cannot overflow),
            row sums via ones-vector matmul (partition reduce on PE),
            reciprocal on DVE, broadcast via K=1 ones matmul
  PV:       out [C, N] = vT-chunks.T @ P^T  (both already in layout)
  proj:     y = WpT-tiles.T @ (out * recip) + pb_eff + x
All big matmuls in float32r (fp32 storage, TF32-like 11-bit-mantissa
multiply, full fp32 PSUM accumulation, 1 cycle/row).

kernel(**inputs) takes the FULL unsharded inputs and returns the full output.
"""
import numpy as np

import concourse.bacc as bacc
import concourse.tile as tile
from concourse import mybir
from concourse.bass_utils import run_bass_kernel_spmd

f32 = mybir.dt.float32
f32r = mybir.dt.float32r
AF = mybir.ActivationFunctionType
ALU = mybir.AluOpType

B, C, H, W = 16, 512, 32, 32
N = H * W                  # 1024
NCORES = 8
BPC = B // NCORES          # 2 batch elements per core
NG = 32                    # groups
GS = C // NG               # 16 channels per group
EPS = 1e-6
NCT = C // 128             # 4 channel tiles
NNT = N // 128             # 8 position tiles
NCHK = N // 512            # 2 free-dim chunks of 512
SCALE = float(C) ** -0.5


def _emit(nc, n_bodies, opts=None):
    opts = opts or {}
    """Emit the kernel body. n_bodies batch-bodies are emitted cycling over
    the BPC batch slots (n_bodies == BPC for the real kernel; larger values
    are used only to build timing-amplified variants)."""
    x_d = nc.declare_dram_parameter("x", [BPC, C, N], f32, isOutput=False)
    wa_d = nc.declare_dram_parameter("wa", [C, C], f32r, isOutput=False)
    wv_d = nc.declare_dram_parameter("wv", [C, C], f32r, isOutput=False)
    wp_d = nc.declare_dram_parameter("wp", [C, C], f32r, isOutput=False)
    gns_d = nc.declare_dram_parameter("gns", [128, NCT], f32, isOutput=False)
    gnb_d = nc.declare_dram_parameter("gnb", [128, NCT], f32, isOutput=False)
    pbe_d = nc.declare_dram_parameter("pbe", [128, NCT], f32, isOutput=False)
    g_d = nc.declare_dram_parameter("gsel", [128, 8], f32, isOutput=False)
    gt_d = nc.declare_dram_parameter("gselT", [8, 128], f32, isOutput=False)
    y_d = nc.declare_dram_parameter("y", [BPC, C, N], f32, isOutput=True)

    from contextlib import ExitStack
    with tile.TileContext(nc) as tc, ExitStack() as ctx:
        sing = ctx.enter_context(tc.tile_pool(name="sing", bufs=1))
        big = ctx.enter_context(tc.tile_pool(name="big", bufs=1))
        gnp = ctx.enter_context(tc.tile_pool(name="gnp", bufs=opts.get("gnp_bufs", 2)))
        mp = ctx.enter_context(tc.tile_pool(name="mp", bufs=6, space="PSUM"))
        stp = ctx.enter_context(tc.tile_pool(name="stp", bufs=2, space="PSUM"))
        sump = stp
        rbp = stp

        # ---- body-0 x loads first (the GN pipeline needs them before any
        # weights are touched) ----
        x_first = [big.tile([128, N], f32, tag=f"x{ct}", name=f"x0_{ct}", bufs=2)
                   for ct in range(NCT)]
        # ---- persistent weights / constants ----
        wat = [sing.tile([128, C], f32r, tag=f"wa{kt}", name=f"wa{kt}")
               for kt in range(NCT)]
        wvt = [sing.tile([128, C], f32r, tag=f"wv{kt}", name=f"wv{kt}")
               for kt in range(NCT)]
        wpt = [sing.tile([128, C], f32r, tag=f"wp{kt}", name=f"wp{kt}")
               for kt in range(NCT)]
        for ct in range(NCT):
            for sg in range(2):
                nc.sync.dma_start(
                    out=x_first[ct][:, sg * 512:(sg + 1) * 512],
                    in_=x_d[0, ct * 128:(ct + 1) * 128, sg * 512:(sg + 1) * 512])
        for kt in range(NCT):
            nc.sync.dma_start(out=wat[kt], in_=wa_d[kt * 128:(kt + 1) * 128, :])
        for kt in range(NCT):
            nc.sync.dma_start(out=wvt[kt], in_=wv_d[kt * 128:(kt + 1) * 128, :])
        gns = sing.tile([128, NCT], f32, tag="gns", name="gns")
        gnb = sing.tile([128, NCT], f32, tag="gnb", name="gnb")
        pbe = sing.tile([128, NCT], f32, tag="pbe", name="pbe")
        g_t = sing.tile([128, 8], f32, tag="g_t", name="g_t")
        gt_t = sing.tile([8, 128], f32, tag="gt_t", name="gt_t")
        nc.gpsimd.dma_start(out=gns, in_=gns_d[:, :])
        nc.gpsimd.dma_start(out=gnb, in_=gnb_d[:, :])
        nc.gpsimd.dma_start(out=pbe, in_=pbe_d[:, :])
        nc.gpsimd.dma_start(out=g_t, in_=g_d[:, :])
        nc.gpsimd.dma_start(out=gt_t, in_=gt_d[:, :])
        for kt in range(NCT):
            nc.sync.dma_start(out=wpt[kt], in_=wp_d[kt * 128:(kt + 1) * 128, :])
        eps_t = sing.tile([128, 1], f32, tag="eps", name="eps")
        nc.vector.memset(eps_t, EPS)
        warm = sing.tile([1, 1], f32, tag="warm", name="warm")
        nc.vector.memset(warm, 1.0)
        nc.scalar.activation(out=warm, in_=warm, func=AF.Exp, scale=1.0)
        nc.scalar.activation(out=warm, in_=warm, func=AF.Sqrt, scale=1.0)
        nc.scalar.activation(out=warm, in_=warm, func=AF.Identity, scale=1.0)
        ones_f32 = sing.tile([128, 1], f32, tag="ones_f", name="ones_f")
        nc.vector.memset(ones_f32, 1.0)
        ones_row_f32 = sing.tile([1, 128], f32, tag="ones_rf", name="ones_rf")
        nc.vector.memset(ones_row_f32, 1.0)
        ones_col = sing.tile([128, 1], f32r, tag="ones_c", name="ones_c")
        nc.vector.tensor_copy(out=ones_col, in_=ones_f32)
        ones_row = sing.tile([1, 128], f32r, tag="ones_r", name="ones_r")
        nc.vector.tensor_copy(out=ones_row, in_=ones_row_f32)

        st8 = {}   # per-body stage state

        def gn_stage(body):
            b = body % BPC
            if body == 0:
                x_t = x_first
            else:
                x_t = [big.tile([128, N], f32, tag=f"x{ct}",
                                name=f"x{body}_{ct}", bufs=2)
                       for ct in range(NCT)]
                for ct in range(NCT):
                    for sg in range(2):
                        nc.sync.dma_start(
                            out=x_t[ct][:, sg * 512:(sg + 1) * 512],
                            in_=x_d[b, ct * 128:(ct + 1) * 128,
                                    sg * 512:(sg + 1) * 512])
            h_t = []
            h_bufs = opts.get("h_bufs", 2)
            for ct in range(NCT):
                st = gnp.tile([128, 2, 6], f32, tag="st", name=f"st{body}_{ct}")
                for sg in range(2):
                    nc.vector.bn_stats(out=st[:, sg, :],
                                       in_=x_t[ct][:, sg * 512:(sg + 1) * 512])
                mv = gnp.tile([128, 2], f32, tag="mv", name=f"mv{body}_{ct}")
                nc.vector.bn_aggr(out=mv, in_=st)
                m1 = gnp.tile([128, 2], f32, tag="m1", name=f"m1{body}_{ct}")
                nc.vector.tensor_copy(out=m1[:, 0:1], in_=mv[:, 0:1])
                sqm = gnp.tile([128, 1], f32, tag="sqm", name=f"sqm{body}_{ct}")
                nc.vector.tensor_mul(out=sqm, in0=mv[:, 0:1], in1=mv[:, 0:1])
                nc.vector.tensor_add(out=m1[:, 1:2], in0=mv[:, 1:2], in1=sqm)
                gs_ps = stp.tile([8, 2], f32, tag="small", name=f"gs{body}_{ct}")
                nc.tensor.matmul(gs_ps, g_t, m1, start=True, stop=True)
                gsb = gnp.tile([8, 2], f32, tag="gsb", name=f"gsb{body}_{ct}")
                nc.scalar.mul(out=gsb, in_=gs_ps, mul=1.0 / GS)
                t8 = gnp.tile([8, 1], f32, tag="t8", name=f"t8{body}_{ct}")
                nc.vector.tensor_mul(out=t8, in0=gsb[:, 0:1], in1=gsb[:, 0:1])
                vg = gnp.tile([8, 1], f32, tag="vg", name=f"vg{body}_{ct}")
                nc.vector.tensor_sub(out=vg, in0=gsb[:, 1:2], in1=t8)
                nc.scalar.activation(out=vg, in_=vg, func=AF.Sqrt,
                                     bias=eps_t[:8, :], scale=1.0)
                st2 = gnp.tile([8, 2], f32, tag="st2", name=f"st2{body}_{ct}")
                nc.vector.tensor_copy(out=st2[:, 0:1], in_=gsb[:, 0:1])
                nc.vector.reciprocal(out=st2[:, 1:2], in_=vg)
                bc_ps = stp.tile([128, 2], f32, tag="small", name=f"bc{body}_{ct}")
                nc.tensor.matmul(bc_ps, gt_t, st2, start=True, stop=True)
                a_sb = gnp.tile([128, 1], f32, tag="a_sb", name=f"a{body}_{ct}")
                nc.vector.tensor_mul(out=a_sb, in0=bc_ps[:, 1:2],
                                     in1=gns[:, ct:ct + 1])
                t1 = gnp.tile([128, 1], f32, tag="t1", name=f"t1{body}_{ct}")
                nc.vector.tensor_mul(out=t1, in0=bc_ps[:, 0:1], in1=a_sb)
                b_sb = gnp.tile([128, 1], f32, tag="b_sb", name=f"bb{body}_{ct}")
                nc.vector.tensor_sub(out=b_sb, in0=gnb[:, ct:ct + 1], in1=t1)
                ht = big.tile([128, N], f32r, tag=f"h{ct}", name=f"h{body}_{ct}",
                              bufs=h_bufs)
                if ct % 2 == 0:
                    nc.vector.tensor_scalar(out=ht, in0=x_t[ct], scalar1=a_sb,
                                            scalar2=b_sb, op0=ALU.mult,
                                            op1=ALU.add)
                else:
                    nc.scalar.activation(out=ht, in_=x_t[ct], func=AF.Identity,
                                         bias=b_sb, scale=a_sb)
                h_t.append(ht)
            st8[body] = {"x_t": x_t, "h_t": h_t}

        def qkv_stage(body):
            h_t = st8[body]["h_t"]
            u_t = [big.tile([128, N], f32r, tag=f"u{ct}", name=f"u{body}_{ct}")
                   for ct in range(NCT)]
            for o in range(NCT):
                for nch in range(NCHK):
                    sl = slice(nch * 512, (nch + 1) * 512)
                    ps = mp.tile([128, 512], f32, tag="mm",
                                 name=f"u{body}_{o}_{nch}")
                    for kt in range(NCT):
                        nc.tensor.matmul(ps, wat[kt][:, o * 128:(o + 1) * 128],
                                         h_t[kt][:, sl],
                                         start=(kt == 0), stop=(kt == NCT - 1))
                    nc.vector.tensor_copy(out=u_t[o][:, sl], in_=ps)
            vT_t = [big.tile([128, C], f32r, tag=f"vT{nt}", name=f"vT{body}_{nt}")
                    for nt in range(NNT)]
            for nt in range(NNT):
                ps = mp.tile([128, 512], f32, tag="mm", name=f"v{body}_{nt}")
                for kt in range(NCT):
                    nc.tensor.matmul(ps, h_t[kt][:, nt * 128:(nt + 1) * 128],
                                     wvt[kt],
                                     start=(kt == 0), stop=(kt == NCT - 1))
                if opts.get("vt_act"):
                    nc.scalar.copy(out=vT_t[nt], in_=ps)
                else:
                    nc.vector.tensor_copy(out=vT_t[nt], in_=ps)
            st8[body].update(u_t=u_t, vT_t=vT_t)

        def sc_stage(body):
            h_t, u_t = st8[body]["h_t"], st8[body]["u_t"]
            pT_t = [big.tile([128, N], f32r, tag=f"pT{mt}", name=f"pT{body}_{mt}")
                    for mt in range(NNT)]
            for mt in range(NNT):
                for nch in range(NCHK):
                    sl = slice(nch * 512, (nch + 1) * 512)
                    ps = mp.tile([128, 512], f32, tag="mm",
                                 name=f"s{body}_{mt}_{nch}")
                    for kt in range(NCT):
                        nc.tensor.matmul(ps, u_t[kt][:, mt * 128:(mt + 1) * 128],
                                         h_t[kt][:, sl],
                                         start=(kt == 0), stop=(kt == NCT - 1))
                    nc.scalar.activation(out=pT_t[mt][:, sl], in_=ps,
                                         func=AF.Exp, scale=SCALE)
            st8[body]["pT_t"] = pT_t

        def sum_stage(body):
            pT_t = st8[body]["pT_t"]
            rb_sb = []
            for nch in range(NCHK):
                sl = slice(nch * 512, (nch + 1) * 512)
                sum_ps = sump.tile([1, 512], f32, tag="small",
                                   name=f"sm{body}_{nch}")
                for mt in range(NNT):
                    nc.tensor.matmul(sum_ps, ones_col, pT_t[mt][:, sl],
                                     start=(mt == 0), stop=(mt == NNT - 1))
                rc = gnp.tile([1, 512], f32r, tag="rc", name=f"rc{body}_{nch}")
                with nc.allow_low_precision(reason="f32r feed for bcast matmul"):
                    nc.vector.reciprocal(out=rc, in_=sum_ps)
                rb_ps = rbp.tile([128, 512], f32, tag="small",
                                 name=f"rbp{body}_{nch}")
                nc.tensor.matmul(rb_ps, ones_row, rc, start=True, stop=True)
                rb = gnp.tile([128, 512], f32, tag="rb_sb", name=f"rb{body}_{nch}")
                nc.vector.tensor_copy(out=rb, in_=rb_ps)
                rb_sb.append(rb)
            st8[body]["rb_sb"] = rb_sb

        def pv_stage(body):
            vT_t, pT_t, rb_sb = (st8[body][k] for k in ("vT_t", "pT_t", "rb_sb"))
            out_t = [big.tile([128, N], f32r, tag=f"o{ct}", name=f"o{body}_{ct}")
                     for ct in range(NCT)]
            for ct in range(NCT):
                for nch in range(NCHK):
                    sl = slice(nch * 512, (nch + 1) * 512)
                    ps = mp.tile([128, 512], f32, tag="mm",
                                 name=f"pv{body}_{ct}_{nch}")
                    for mt in range(NNT):
                        nc.tensor.matmul(ps,
                                         vT_t[mt][:, ct * 128:(ct + 1) * 128],
                                         pT_t[mt][:, sl],
                                         start=(mt == 0), stop=(mt == NNT - 1))
                    nc.vector.tensor_mul(out=out_t[ct][:, sl], in0=ps,
                                         in1=rb_sb[nch])
            st8[body]["out_t"] = out_t

        def proj_stage(body):
            b = body % BPC
            out_t, x_t = st8[body]["out_t"], st8[body]["x_t"]
            for ot in range(NCT):
                for nch in range(NCHK):
                    sl = slice(nch * 512, (nch + 1) * 512)
                    ps = mp.tile([128, 512], f32, tag="mm",
                                 name=f"pj{body}_{ot}_{nch}")
                    for ct in range(NCT):
                        nc.tensor.matmul(ps, wpt[ct][:, ot * 128:(ot + 1) * 128],
                                         out_t[ct][:, sl],
                                         start=(ct == 0), stop=(ct == NCT - 1))
                    fc = gnp.tile([128, 512], f32, tag="fin",
                                  bufs=opts.get("fin_bufs", 6),
                                  name=f"fin{body}_{ot}_{nch}")
                    nc.scalar.activation(out=fc, in_=ps, func=AF.Identity,
                                         bias=pbe[:, ot:ot + 1], scale=1.0)
                    nc.vector.tensor_add(out=fc, in0=fc, in1=x_t[ot][:, sl])
                    nc.sync.dma_start(
                        out=y_d[b, ot * 128:(ot + 1) * 128, sl], in_=fc)
            st8[body].clear()

        ilv = opts.get("interleave", 1)
        if ilv == 2:
            gn_stage(0); qkv_stage(0)
            for k in range(n_bodies):
                if k + 1 < n_bodies:
                    gn_stage(k + 1)
                sc_stage(k); sum_stage(k); pv_stage(k)
                if k + 1 < n_bodies:
                    qkv_stage(k + 1)
                proj_stage(k)
        elif ilv:
            gn_stage(0); qkv_stage(0); sc_stage(0)
            for k in range(n_bodies):
                if k + 1 < n_bodies:
                    gn_stage(k + 1)
                sum_stage(k); pv_stage(k)
                if k + 1 < n_bodies:
                    qkv_stage(k + 1)
                proj_stage(k)
                if k + 1 < n_bodies:
                    sc_stage(k + 1)
        else:
            for k in range(n_bodies):
                gn_stage(k); qkv_stage(k); sc_stage(k)
                sum_stage(k); pv_stage(k); proj_stage(k)


def build(n_bodies=BPC, **opts):
    nc = bacc.Bacc("TRN2")
    _emit(nc, n_bodies, opts)
    nc.compile()
    return nc


_cached = {}


def get_nc(n_bodies=BPC, **opts):
    key = (n_bodies, tuple(sorted(opts.items())))
    if key not in _cached:
        _cached[key] = build(n_bodies, **opts)
    return _cached[key]


def make_in_maps(x, gn_scale, gn_bias, qkv_w, qkv_b, proj_w, proj_b):
    x = np.ascontiguousarray(np.asarray(x, np.float32).reshape(B, C, N))
    gn_scale = np.asarray(gn_scale, np.float32)
    gn_bias = np.asarray(gn_bias, np.float32)
    qkv_w = np.asarray(qkv_w, np.float32)
    qkv_b = np.asarray(qkv_b, np.float32)
    proj_w = np.asarray(proj_w, np.float32)
    proj_b = np.asarray(proj_b, np.float32)

    assert np.abs(qkv_b[:2 * C]).max() == 0.0, "q/k biases assumed zero"
    wq = qkv_w[0:C].astype(np.float64)        # [C, C] rows o, cols c
    wk = qkv_w[C:2 * C].astype(np.float64)
    A = wq.T @ wk                             # [C(c'), C(c)]; S = h^T A h
    waT = np.ascontiguousarray(A.T.astype(np.float32))   # lhsT layout [c, c']
    wvT = np.ascontiguousarray(qkv_w[2 * C:].T)          # [C, C]
    wpT = np.ascontiguousarray(proj_w.T)                 # [C, C]
    gns = np.ascontiguousarray(gn_scale.reshape(NCT, 128).T)
    gnb = np.ascontiguousarray(gn_bias.reshape(NCT, 128).T)
    pbe_vec = proj_w @ qkv_b[2 * C:] + proj_b                  # fold v-bias
    pbe = np.ascontiguousarray(pbe_vec.astype(np.float32).reshape(NCT, 128).T)
    gsel = np.zeros((128, 8), np.float32)
    gsel[np.arange(128), np.arange(128) // GS] = 1.0
    gselT = np.ascontiguousarray(gsel.T)

    shared = {"wa": waT, "wv": wvT, "wp": wpT, "gns": gns,
              "gnb": gnb, "pbe": pbe, "gsel": gsel, "gselT": gselT}
    return [{"x": np.ascontiguousarray(x[BPC * i:BPC * (i + 1)]), **shared}
            for i in range(NCORES)]


def kernel(x, gn_scale, gn_bias, qkv_w, qkv_b, proj_w, proj_b):
    in_maps = make_in_maps(x, gn_scale, gn_bias, qkv_w, qkv_b, proj_w, proj_b)
    nc = get_nc()
    res = run_bass_kernel_spmd(nc, in_maps, list(range(NCORES)))
    y = np.concatenate([res.results[i]["y"] for i in range(NCORES)], axis=0)
    return np.ascontiguousarray(y.reshape(B, C, H, W).astype(np.float32))


# revision 24
# speedup vs baseline: 1.0819x; 1.0819x over previous
"""Trainium2 Bass kernel for nn_AttentionBlock (GroupNorm + single-head
attention over N=HW + 1x1 convs + residual).

Sharding: data-parallel over batch. B=16 across 8 cores -> 2 batch elements
per core, no collectives.

Per-core pipeline (per batch element, layouts chosen so no PE transposes are
ever needed):
  x        [C=512(part,4x128), N=1024(free)]  fp32
  GroupNorm: bn_stats/bn_aggr per channel, cross-partition group reduce via a
             tiny fp32 matmul with a 0/1 group-selection matrix, broadcast
             back via its transpose; h = x*a + b  (a,b per-channel) -> f32r
  scores:   q/k biases are zero, so S = h^T (Wq^T Wk) h with A = Wq^T Wk
            precomputed on the host: u = A h (one matmul instead of q AND k),
            then S^T [m, n] = u-tiles.T @ h  -> PSUM
  v:        produced directly transposed, vT [N, C] = h-tiles.T @ WvT
  softmax:  exp on ACT straight out of PSUM (1/sqrt(C) folded into the
            activation scale), no max-subtraction (scores are O(5), fp32 exp
            cannot overflow), row sums via ones-vector matmul (partition
            reduce on PE), reciprocal on DVE, broadcast via a K=1 ones matmul
  PV:       out [C, N] = vT-chunks.T @ P^T  (both already in layout)
  proj:     y = WpT-tiles.T @ (out * recip) + pb_eff + x
            (v-bias and proj-bias folded into pb_eff on the host)
All big matmuls in float32r (fp32 storage, TF32-like 11-bit-mantissa
multiply, full fp32 PSUM accumulation, 1 cycle/row = full bf16-equivalent
throughput, measured 212 ns per [128x128]@[128x512] self-loading MM).
The two batch bodies are emitted software-pipelined (next body's GN/QKV
interleaved into the previous body's softmax/PV/proj) to keep the PE dense
across the body boundary.

kernel(**inputs) takes the FULL unsharded inputs and returns the full output.
"""
import numpy as np

import concourse.bacc as bacc
import concourse.tile as tile
from concourse import mybir
from concourse.bass_utils import run_bass_kernel_spmd

f32 = mybir.dt.float32
f32r = mybir.dt.float32r
AF = mybir.ActivationFunctionType
ALU = mybir.AluOpType

B, C, H, W = 16, 512, 32, 32
N = H * W                  # 1024
NCORES = 8
BPC = B // NCORES          # 2 batch elements per core
NG = 32                    # groups
GS = C // NG               # 16 channels per group
EPS = 1e-6
NCT = C // 128             # 4 channel tiles
NNT = N // 128             # 8 position tiles
NCHK = N // 512            # 2 free-dim chunks of 512
SCALE = float(C) ** -0.5


def _emit(nc, n_bodies, opts=None):
    opts = opts or {}
    """Emit the kernel body. n_bodies batch-bodies are emitted cycling over
    the BPC batch slots (n_bodies == BPC for the real kernel; larger values
    are used only to build timing-amplified variants)."""
    x_d = nc.declare_dram_parameter("x", [BPC, C, N], f32, isOutput=False)
    wa_d = nc.declare_dram_parameter("wa", [C, C], f32r, isOutput=False)
    wv_d = nc.declare_dram_parameter("wv", [C, C], f32r, isOutput=False)
    wp_d = nc.declare_dram_parameter("wp", [C, C], f32r, isOutput=False)
    gns_d = nc.declare_dram_parameter("gns", [128, NCT], f32, isOutput=False)
    gnb_d = nc.declare_dram_parameter("gnb", [128, NCT], f32, isOutput=False)
    pbe_d = nc.declare_dram_parameter("pbe", [128, NCT], f32, isOutput=False)
    g_d = nc.declare_dram_parameter("gsel", [128, 8], f32, isOutput=False)
    gt_d = nc.declare_dram_parameter("gselT", [8, 128], f32, isOutput=False)
    y_d = nc.declare_dram_parameter("y", [BPC, C, N], f32, isOutput=True)

    from contextlib import ExitStack
    with tile.TileContext(nc) as tc, ExitStack() as ctx:
        sing = ctx.enter_context(tc.tile_pool(name="sing", bufs=1))
        big = ctx.enter_context(tc.tile_pool(name="big", bufs=1))
        gnp = ctx.enter_context(tc.tile_pool(name="gnp", bufs=opts.get("gnp_bufs", 2)))
        mp = ctx.enter_context(tc.tile_pool(name="mp", bufs=opts.get("mm_bufs", 6), space="PSUM"))
        stp = ctx.enter_context(tc.tile_pool(name="stp", bufs=opts.get("small_bufs", 2), space="PSUM"))
        sump = stp
        rbp = stp

        # ---- body-0 x loads first (the GN pipeline needs them before any
        # weights are touched) ----
        x_first = [big.tile([128, N], f32, tag=f"x{ct}", name=f"x0_{ct}", bufs=2)
                   for ct in range(NCT)]
        # ---- persistent weights / constants ----
        wat = [sing.tile([128, C], f32r, tag=f"wa{kt}", name=f"wa{kt}")
               for kt in range(NCT)]
        wvt = [sing.tile([128, C], f32r, tag=f"wv{kt}", name=f"wv{kt}")
               for kt in range(NCT)]
        wpt = [sing.tile([128, C], f32r, tag=f"wp{kt}", name=f"wp{kt}")
               for kt in range(NCT)]
        def _xf(ct):
            for sg in range(2):
                nc.sync.dma_start(
                    out=x_first[ct][:, sg * 512:(sg + 1) * 512],
                    in_=x_d[0, ct * 128:(ct + 1) * 128, sg * 512:(sg + 1) * 512])
        def _wa(kt):
            nc.sync.dma_start(out=wat[kt], in_=wa_d[kt * 128:(kt + 1) * 128, :])
        if opts.get("dma_ramp", 1):
            _xf(0); _xf(1); _wa(0); _xf(2); _wa(1); _xf(3); _wa(2); _wa(3)
        else:
            for ct in range(NCT):
                _xf(ct)
            for kt in range(NCT):
                _wa(kt)
        for kt in range(NCT):
            nc.sync.dma_start(out=wvt[kt], in_=wv_d[kt * 128:(kt + 1) * 128, :])
        gns = sing.tile([128, NCT], f32, tag="gns", name="gns")
        gnb = sing.tile([128, NCT], f32, tag="gnb", name="gnb")
        pbe = sing.tile([128, NCT], f32, tag="pbe", name="pbe")
        g_t = sing.tile([128, 8], f32, tag="g_t", name="g_t")
        gt_t = sing.tile([8, 128], f32, tag="gt_t", name="gt_t")
        nc.gpsimd.dma_start(out=gns, in_=gns_d[:, :])
        nc.gpsimd.dma_start(out=gnb, in_=gnb_d[:, :])
        nc.gpsimd.dma_start(out=pbe, in_=pbe_d[:, :])
        nc.gpsimd.dma_start(out=g_t, in_=g_d[:, :])
        nc.gpsimd.dma_start(out=gt_t, in_=gt_d[:, :])
        for kt in range(NCT):
            nc.sync.dma_start(out=wpt[kt], in_=wp_d[kt * 128:(kt + 1) * 128, :])
        eps_t = sing.tile([128, 1], f32, tag="eps", name="eps")
        nc.vector.memset(eps_t, EPS)
        warm = sing.tile([1, 1], f32, tag="warm", name="warm")
        nc.vector.memset(warm, 1.0)
        nc.scalar.activation(out=warm, in_=warm, func=AF.Exp, scale=1.0)
        nc.scalar.activation(out=warm, in_=warm, func=AF.Sqrt, scale=1.0)
        nc.scalar.activation(out=warm, in_=warm, func=AF.Identity, scale=1.0)
        ones_f32 = sing.tile([128, 1], f32, tag="ones_f", name="ones_f")
        nc.vector.memset(ones_f32, 1.0)
        ones_row_f32 = sing.tile([1, 128], f32, tag="ones_rf", name="ones_rf")
        nc.vector.memset(ones_row_f32, 1.0)
        ones_col = sing.tile([128, 1], f32r, tag="ones_c", name="ones_c")
        nc.vector.tensor_copy(out=ones_col, in_=ones_f32)
        ones_row = sing.tile([1, 128], f32r, tag="ones_r", name="ones_r")
        nc.vector.tensor_copy(out=ones_row, in_=ones_row_f32)

        st8 = {}   # per-body stage state

        def gn_stage(body):
            b = body % BPC
            if body == 0:
                x_t = x_first
            else:
                x_t = [big.tile([128, N], f32, tag=f"x{ct}",
                                name=f"x{body}_{ct}", bufs=2)
                       for ct in range(NCT)]
                for ct in range(NCT):
                    for sg in range(2):
                        nc.sync.dma_start(
                            out=x_t[ct][:, sg * 512:(sg + 1) * 512],
                            in_=x_d[b, ct * 128:(ct + 1) * 128,
                                    sg * 512:(sg + 1) * 512])
            h_t = []
            h_bufs = opts.get("h_bufs", 2)
            for ct in range(NCT):
                st = gnp.tile([128, 2, 6], f32, tag="st", name=f"st{body}_{ct}")
                for sg in range(2):
                    nc.vector.bn_stats(out=st[:, sg, :],
                                       in_=x_t[ct][:, sg * 512:(sg + 1) * 512])
                mv = gnp.tile([128, 2], f32, tag="mv", name=f"mv{body}_{ct}")
                nc.vector.bn_aggr(out=mv, in_=st)
                m1 = gnp.tile([128, 2], f32, tag="m1", name=f"m1{body}_{ct}")
                nc.vector.tensor_copy(out=m1[:, 0:1], in_=mv[:, 0:1])
                sqm = gnp.tile([128, 1], f32, tag="sqm", name=f"sqm{body}_{ct}")
                nc.vector.tensor_mul(out=sqm, in0=mv[:, 0:1], in1=mv[:, 0:1])
                nc.vector.tensor_add(out=m1[:, 1:2], in0=mv[:, 1:2], in1=sqm)
                gs_ps = stp.tile([8, 2], f32, tag="small", name=f"gs{body}_{ct}")
                nc.tensor.matmul(gs_ps, g_t, m1, start=True, stop=True)
                gsb = gnp.tile([8, 2], f32, tag="gsb", name=f"gsb{body}_{ct}")
                nc.scalar.mul(out=gsb, in_=gs_ps, mul=1.0 / GS)
                t8 = gnp.tile([8, 1], f32, tag="t8", name=f"t8{body}_{ct}")
                nc.vector.tensor_mul(out=t8, in0=gsb[:, 0:1], in1=gsb[:, 0:1])
                vg = gnp.tile([8, 1], f32, tag="vg", name=f"vg{body}_{ct}")
                nc.vector.tensor_sub(out=vg, in0=gsb[:, 1:2], in1=t8)
                nc.scalar.activation(out=vg, in_=vg, func=AF.Sqrt,
                                     bias=eps_t[:8, :], scale=1.0)
                st2 = gnp.tile([8, 2], f32, tag="st2", name=f"st2{body}_{ct}")
                nc.vector.tensor_copy(out=st2[:, 0:1], in_=gsb[:, 0:1])
                nc.vector.reciprocal(out=st2[:, 1:2], in_=vg)
                bc_ps = stp.tile([128, 2], f32, tag="small", name=f"bc{body}_{ct}")
                nc.tensor.matmul(bc_ps, gt_t, st2, start=True, stop=True)
                a_sb = gnp.tile([128, 1], f32, tag="a_sb", name=f"a{body}_{ct}")
                nc.vector.tensor_mul(out=a_sb, in0=bc_ps[:, 1:2],
                                     in1=gns[:, ct:ct + 1])
                t1 = gnp.tile([128, 1], f32, tag="t1", name=f"t1{body}_{ct}")
                nc.vector.tensor_mul(out=t1, in0=bc_ps[:, 0:1], in1=a_sb)
                b_sb = gnp.tile([128, 1], f32, tag="b_sb", name=f"bb{body}_{ct}")
                nc.vector.tensor_sub(out=b_sb, in0=gnb[:, ct:ct + 1], in1=t1)
                ht = big.tile([128, N], f32r, tag=f"h{ct}", name=f"h{body}_{ct}",
                              bufs=h_bufs)
                if ct % 2 == 0 and not opts.get("h_act_all"):
                    nc.vector.tensor_scalar(out=ht, in0=x_t[ct], scalar1=a_sb,
                                            scalar2=b_sb, op0=ALU.mult,
                                            op1=ALU.add)
                else:
                    nc.scalar.activation(out=ht, in_=x_t[ct], func=AF.Identity,
                                         bias=b_sb, scale=a_sb)
                h_t.append(ht)
            st8[body] = {"x_t": x_t, "h_t": h_t}

        def qkv_stage(body):
            h_t = st8[body]["h_t"]
            u_t = [big.tile([128, N], f32r, tag=f"u{ct}", name=f"u{body}_{ct}")
                   for ct in range(NCT)]
            for o in range(NCT):
                for nch in range(NCHK):
                    sl = slice(nch * 512, (nch + 1) * 512)
                    ps = mp.tile([128, 512], f32, tag="mm",
                                 name=f"u{body}_{o}_{nch}")
                    for kt in range(NCT):
                        nc.tensor.matmul(ps, wat[kt][:, o * 128:(o + 1) * 128],
                                         h_t[kt][:, sl],
                                         start=(kt == 0), stop=(kt == NCT - 1))
                    nc.vector.tensor_copy(out=u_t[o][:, sl], in_=ps)
            vT_t = [big.tile([128, C], f32r, tag=f"vT{nt}", name=f"vT{body}_{nt}")
                    for nt in range(NNT)]
            for nt in range(NNT):
                ps = mp.tile([128, 512], f32, tag="mm", name=f"v{body}_{nt}")
                for kt in range(NCT):
                    nc.tensor.matmul(ps, h_t[kt][:, nt * 128:(nt + 1) * 128],
                                     wvt[kt],
                                     start=(kt == 0), stop=(kt == NCT - 1))
                if opts.get("vt_act"):
                    nc.scalar.copy(out=vT_t[nt], in_=ps)
                else:
                    nc.vector.tensor_copy(out=vT_t[nt], in_=ps)
            st8[body].update(u_t=u_t, vT_t=vT_t)

        def sc_stage(body):
            h_t, u_t = st8[body]["h_t"], st8[body]["u_t"]
            pT_t = [big.tile([128, N], f32r, tag=f"pT{mt}", name=f"pT{body}_{mt}")
                    for mt in range(NNT)]
            for mt in range(NNT):
                for nch in range(NCHK):
                    sl = slice(nch * 512, (nch + 1) * 512)
                    ps = mp.tile([128, 512], f32, tag="mm",
                                 name=f"s{body}_{mt}_{nch}")
                    for kt in range(NCT):
                        nc.tensor.matmul(ps, u_t[kt][:, mt * 128:(mt + 1) * 128],
                                         h_t[kt][:, sl],
                                         start=(kt == 0), stop=(kt == NCT - 1))
                    nc.scalar.activation(out=pT_t[mt][:, sl], in_=ps,
                                         func=AF.Exp, scale=SCALE)
            st8[body]["pT_t"] = pT_t

        def sum_stage(body):
            pT_t = st8[body]["pT_t"]
            rb_sb = []
            for nch in range(NCHK):
                sl = slice(nch * 512, (nch + 1) * 512)
                sum_ps = sump.tile([1, 512], f32, tag="small",
                                   name=f"sm{body}_{nch}")
                for mt in range(NNT):
                    nc.tensor.matmul(sum_ps, ones_col, pT_t[mt][:, sl],
                                     start=(mt == 0), stop=(mt == NNT - 1))
                rc = gnp.tile([1, 512], f32r, tag="rc", name=f"rc{body}_{nch}")
                with nc.allow_low_precision(reason="f32r feed for bcast matmul"):
                    nc.vector.reciprocal(out=rc, in_=sum_ps)
                rb_ps = rbp.tile([128, 512], f32, tag="small",
                                 name=f"rbp{body}_{nch}")
                nc.tensor.matmul(rb_ps, ones_row, rc, start=True, stop=True)
                rb = gnp.tile([128, 512], f32, tag="rb_sb", name=f"rb{body}_{nch}")
                nc.vector.tensor_copy(out=rb, in_=rb_ps)
                rb_sb.append(rb)
            st8[body]["rb_sb"] = rb_sb

        def pv_stage(body):
            vT_t, pT_t, rb_sb = (st8[body][k] for k in ("vT_t", "pT_t", "rb_sb"))
            out_t = [big.tile([128, N], f32r, tag=f"o{ct}", name=f"o{body}_{ct}")
                     for ct in range(NCT)]
            for ct in range(NCT):
                for nch in range(NCHK):
                    sl = slice(nch * 512, (nch + 1) * 512)
                    ps = mp.tile([128, 512], f32, tag="mm",
                                 name=f"pv{body}_{ct}_{nch}")
                    for mt in range(NNT):
                        nc.tensor.matmul(ps,
                                         vT_t[mt][:, ct * 128:(ct + 1) * 128],
                                         pT_t[mt][:, sl],
                                         start=(mt == 0), stop=(mt == NNT - 1))
                    nc.vector.tensor_mul(out=out_t[ct][:, sl], in0=ps,
                                         in1=rb_sb[nch])
            st8[body]["out_t"] = out_t

        def proj_stage(body):
            b = body % BPC
            out_t, x_t = st8[body]["out_t"], st8[body]["x_t"]
            for ot in range(NCT):
                for nch in range(NCHK):
                    sl = slice(nch * 512, (nch + 1) * 512)
                    ps = mp.tile([128, 512], f32, tag="mm",
                                 name=f"pj{body}_{ot}_{nch}")
                    for ct in range(NCT):
                        nc.tensor.matmul(ps, wpt[ct][:, ot * 128:(ot + 1) * 128],
                                         out_t[ct][:, sl],
                                         start=(ct == 0), stop=(ct == NCT - 1))
                    fc = gnp.tile([128, 512], f32, tag="fin",
                                  bufs=opts.get("fin_bufs", 6),
                                  name=f"fin{body}_{ot}_{nch}")
                    nc.scalar.activation(out=fc, in_=ps, func=AF.Identity,
                                         bias=pbe[:, ot:ot + 1], scale=1.0)
                    nc.vector.tensor_add(out=fc, in0=fc, in1=x_t[ot][:, sl])
                    nc.sync.dma_start(
                        out=y_d[b, ot * 128:(ot + 1) * 128, sl], in_=fc)
            st8[body].clear()

        ilv = opts.get("interleave", 1)
        if ilv == 2:
            gn_stage(0); qkv_stage(0)
            for k in range(n_bodies):
                if k + 1 < n_bodies:
                    gn_stage(k + 1)
                sc_stage(k); sum_stage(k); pv_stage(k)
                if k + 1 < n_bodies:
                    qkv_stage(k + 1)
                proj_stage(k)
        elif ilv:
            gn_stage(0); qkv_stage(0); sc_stage(0)
            for k in range(n_bodies):
                if k + 1 < n_bodies:
                    gn_stage(k + 1)
                sum_stage(k); pv_stage(k)
                if k + 1 < n_bodies:
                    qkv_stage(k + 1)
                proj_stage(k)
                if k + 1 < n_bodies:
                    sc_stage(k + 1)
        else:
            for k in range(n_bodies):
                gn_stage(k); qkv_stage(k); sc_stage(k)
                sum_stage(k); pv_stage(k); proj_stage(k)


def build(n_bodies=BPC, **opts):
    nc = bacc.Bacc("TRN2")
    _emit(nc, n_bodies, opts)
    nc.compile()
    return nc


_cached = {}


def get_nc(n_bodies=BPC, **opts):
    key = (n_bodies, tuple(sorted(opts.items())))
    if key not in _cached:
        _cached[key] = build(n_bodies, **opts)
    return _cached[key]


def make_in_maps(x, gn_scale, gn_bias, qkv_w, qkv_b, proj_w, proj_b):
    x = np.ascontiguousarray(np.asarray(x, np.float32).reshape(B, C, N))
    gn_scale = np.asarray(gn_scale, np.float32)
    gn_bias = np.asarray(gn_bias, np.float32)
    qkv_w = np.asarray(qkv_w, np.float32)
    qkv_b = np.asarray(qkv_b, np.float32)
    proj_w = np.asarray(proj_w, np.float32)
    proj_b = np.asarray(proj_b, np.float32)

    assert np.abs(qkv_b[:2 * C]).max() == 0.0, "q/k biases assumed zero"
    wq = qkv_w[0:C].astype(np.float64)        # [C, C] rows o, cols c
    wk = qkv_w[C:2 * C].astype(np.float64)
    A = wq.T @ wk                             # [C(c'), C(c)]; S = h^T A h
    waT = np.ascontiguousarray(A.T.astype(np.float32))   # lhsT layout [c, c']
    wvT = np.ascontiguousarray(qkv_w[2 * C:].T)          # [C, C]
    wpT = np.ascontiguousarray(proj_w.T)                 # [C, C]
    gns = np.ascontiguousarray(gn_scale.reshape(NCT, 128).T)
    gnb = np.ascontiguousarray(gn_bias.reshape(NCT, 128).T)
    pbe_vec = proj_w @ qkv_b[2 * C:] + proj_b                  # fold v-bias
    pbe = np.ascontiguousarray(pbe_vec.astype(np.float32).reshape(NCT, 128).T)
    gsel = np.zeros((128, 8), np.float32)
    gsel[np.arange(128), np.arange(128) // GS] = 1.0
    gselT = np.ascontiguousarray(gsel.T)

    shared = {"wa": waT, "wv": wvT, "wp": wpT, "gns": gns,
              "gnb": gnb, "pbe": pbe, "gsel": gsel, "gselT": gselT}
    return [{"x": np.ascontiguousarray(x[BPC * i:BPC * (i + 1)]), **shared}
            for i in range(NCORES)]


def kernel(x, gn_scale, gn_bias, qkv_w, qkv_b, proj_w, proj_b):
    in_maps = make_in_maps(x, gn_scale, gn_bias, qkv_w, qkv_b, proj_w, proj_b)
    nc = get_nc()
    res = run_bass_kernel_spmd(nc, in_maps, list(range(NCORES)))
    y = np.concatenate([res.results[i]["y"] for i in range(NCORES)], axis=0)
    return np.ascontiguousarray(y.reshape(B, C, H, W).astype(np.float32))


# revision 25
# speedup vs baseline: 1.0940x; 1.0112x over previous
"""Trainium2 Bass kernel for nn_AttentionBlock (GroupNorm + single-head
attention over N=HW + 1x1 convs + residual).

Sharding: data-parallel over batch. B=16 across 8 cores -> 2 batch elements
per core, no collectives.

Per-core pipeline (per batch element, layouts chosen so no PE transposes are
ever needed):
  x        [C=512(part,4x128), N=1024(free)]  fp32
  GroupNorm: bn_stats/bn_aggr per channel, cross-partition group reduce via a
             tiny fp32 matmul with a 0/1 group-selection matrix, broadcast
             back via its transpose; h = x*a + b  (a,b per-channel) -> f32r
  scores:   q/k biases are zero, so S = h^T (Wq^T Wk) h with A = Wq^T Wk
            precomputed on the host: u = A h (one matmul instead of q AND k),
            then S^T [m, n] = u-tiles.T @ h  -> PSUM
  v:        produced directly transposed, vT [N, C] = h-tiles.T @ WvT
  softmax:  exp on ACT straight out of PSUM (1/sqrt(C) folded into the
            activation scale), no max-subtraction (scores are O(5), fp32 exp
            cannot overflow), row sums via ones-vector matmul (partition
            reduce on PE), reciprocal on DVE, broadcast via a K=1 ones matmul
  PV:       out [C, N] = vT-chunks.T @ P^T  (both already in layout)
  proj:     y = WpT-tiles.T @ (out * recip) + pb_eff + x
            (v-bias and proj-bias folded into pb_eff on the host)
All big matmuls in float32r (fp32 storage, TF32-like 11-bit-mantissa
multiply, full fp32 PSUM accumulation, 1 cycle/row = full bf16-equivalent
throughput, measured 212 ns per [128x128]@[128x512] self-loading MM).
The two batch bodies are emitted software-pipelined (next body's GN/QKV
interleaved into the previous body's softmax/PV/proj) to keep the PE dense
across the body boundary.

kernel(**inputs) takes the FULL unsharded inputs and returns the full output.
"""
import numpy as np

import concourse.bacc as bacc
import concourse.tile as tile
from concourse import mybir
from concourse.bass_utils import run_bass_kernel_spmd

f32 = mybir.dt.float32
f32r = mybir.dt.float32r
AF = mybir.ActivationFunctionType
ALU = mybir.AluOpType

B, C, H, W = 16, 512, 32, 32
N = H * W                  # 1024
NCORES = 8
BPC = B // NCORES          # 2 batch elements per core
NG = 32                    # groups
GS = C // NG               # 16 channels per group
EPS = 1e-6
NCT = C // 128             # 4 channel tiles
NNT = N // 128             # 8 position tiles
NCHK = N // 512            # 2 free-dim chunks of 512
SCALE = float(C) ** -0.5


def _emit(nc, n_bodies, opts=None):
    opts = opts or {}
    """Emit the kernel body. n_bodies batch-bodies are emitted cycling over
    the BPC batch slots (n_bodies == BPC for the real kernel; larger values
    are used only to build timing-amplified variants)."""
    x_d = nc.declare_dram_parameter("x", [BPC, C, N], f32, isOutput=False)
    wa_d = nc.declare_dram_parameter("wa", [C, C], f32r, isOutput=False)
    wv_d = nc.declare_dram_parameter("wv", [C, C], f32r, isOutput=False)
    wp_d = nc.declare_dram_parameter("wp", [C, C], f32r, isOutput=False)
    gns_d = nc.declare_dram_parameter("gns", [128, NCT], f32, isOutput=False)
    gnb_d = nc.declare_dram_parameter("gnb", [128, NCT], f32, isOutput=False)
    pbe_d = nc.declare_dram_parameter("pbe", [128, NCT], f32, isOutput=False)
    g_d = nc.declare_dram_parameter("gsel", [128, 8], f32, isOutput=False)
    gt_d = nc.declare_dram_parameter("gselT", [8, 128], f32, isOutput=False)
    y_d = nc.declare_dram_parameter("y", [BPC, C, N], f32, isOutput=True)

    from contextlib import ExitStack
    with tile.TileContext(nc) as tc, ExitStack() as ctx:
        sing = ctx.enter_context(tc.tile_pool(name="sing", bufs=1))
        big = ctx.enter_context(tc.tile_pool(name="big", bufs=1))
        gnp = ctx.enter_context(tc.tile_pool(name="gnp", bufs=opts.get("gnp_bufs", 2)))
        mp = ctx.enter_context(tc.tile_pool(name="mp", bufs=opts.get("mm_bufs", 6), space="PSUM"))
        stp = ctx.enter_context(tc.tile_pool(name="stp", bufs=opts.get("small_bufs", 2), space="PSUM"))
        sump = stp
        rbp = stp

        # ---- body-0 x loads first (the GN pipeline needs them before any
        # weights are touched) ----
        x_first = [big.tile([128, N], f32, tag=f"x{ct}", name=f"x0_{ct}", bufs=2)
                   for ct in range(NCT)]
        # ---- persistent weights / constants ----
        wat = [sing.tile([128, C], f32r, tag=f"wa{kt}", name=f"wa{kt}")
               for kt in range(NCT)]
        wvt = [sing.tile([128, C], f32r, tag=f"wv{kt}", name=f"wv{kt}")
               for kt in range(NCT)]
        wpt = [sing.tile([128, C], f32r, tag=f"wp{kt}", name=f"wp{kt}")
               for kt in range(NCT)]
        def _xf(ct):
            for sg in range(2):
                nc.sync.dma_start(
                    out=x_first[ct][:, sg * 512:(sg + 1) * 512],
                    in_=x_d[0, ct * 128:(ct + 1) * 128, sg * 512:(sg + 1) * 512])
        def _wa(kt):
            nc.sync.dma_start(out=wat[kt], in_=wa_d[kt * 128:(kt + 1) * 128, :])
        if opts.get("dma_ramp", 1):
            _xf(0); _xf(1); _wa(0); _xf(2); _wa(1); _xf(3); _wa(2); _wa(3)
        else:
            for ct in range(NCT):
                _xf(ct)
            for kt in range(NCT):
                _wa(kt)
        for kt in range(NCT):
            nc.sync.dma_start(out=wvt[kt], in_=wv_d[kt * 128:(kt + 1) * 128, :])
        gns = sing.tile([128, NCT], f32, tag="gns", name="gns")
        gnb = sing.tile([128, NCT], f32, tag="gnb", name="gnb")
        pbe = sing.tile([128, NCT], f32, tag="pbe", name="pbe")
        g_t = sing.tile([128, 8], f32, tag="g_t", name="g_t")
        gt_t = sing.tile([8, 128], f32, tag="gt_t", name="gt_t")
        nc.gpsimd.dma_start(out=gns, in_=gns_d[:, :])
        nc.gpsimd.dma_start(out=gnb, in_=gnb_d[:, :])
        nc.gpsimd.dma_start(out=pbe, in_=pbe_d[:, :])
        nc.gpsimd.dma_start(out=g_t, in_=g_d[:, :])
        nc.gpsimd.dma_start(out=gt_t, in_=gt_d[:, :])
        for kt in range(NCT):
            nc.sync.dma_start(out=wpt[kt], in_=wp_d[kt * 128:(kt + 1) * 128, :])
        eps_t = sing.tile([128, 1], f32, tag="eps", name="eps")
        nc.vector.memset(eps_t, EPS)
        warm = sing.tile([1, 1], f32, tag="warm", name="warm")
        nc.vector.memset(warm, 1.0)
        nc.scalar.activation(out=warm, in_=warm, func=AF.Exp, scale=1.0)
        nc.scalar.activation(out=warm, in_=warm, func=AF.Sqrt, scale=1.0)
        nc.scalar.activation(out=warm, in_=warm, func=AF.Identity, scale=1.0)
        ones_f32 = sing.tile([128, 1], f32, tag="ones_f", name="ones_f")
        nc.vector.memset(ones_f32, 1.0)
        ones_row_f32 = sing.tile([1, 128], f32, tag="ones_rf", name="ones_rf")
        nc.vector.memset(ones_row_f32, 1.0)
        ones_col = sing.tile([128, 1], f32r, tag="ones_c", name="ones_c")
        nc.vector.tensor_copy(out=ones_col, in_=ones_f32)
        ones_row = sing.tile([1, 128], f32r, tag="ones_r", name="ones_r")
        nc.vector.tensor_copy(out=ones_row, in_=ones_row_f32)

        st8 = {}   # per-body stage state

        def gn_stage(body):
            b = body % BPC
            if body == 0:
                x_t = x_first
            else:
                x_t = [big.tile([128, N], f32, tag=f"x{ct}",
                                name=f"x{body}_{ct}", bufs=2)
                       for ct in range(NCT)]
                for ct in range(NCT):
                    for sg in range(2):
                        nc.sync.dma_start(
                            out=x_t[ct][:, sg * 512:(sg + 1) * 512],
                            in_=x_d[b, ct * 128:(ct + 1) * 128,
                                    sg * 512:(sg + 1) * 512])
            h_t = []
            h_bufs = opts.get("h_bufs", 2)
            for ct in range(NCT):
                st = gnp.tile([128, 2, 6], f32, tag="st", name=f"st{body}_{ct}")
                for sg in range(2):
                    nc.vector.bn_stats(out=st[:, sg, :],
                                       in_=x_t[ct][:, sg * 512:(sg + 1) * 512])
                mv = gnp.tile([128, 2], f32, tag="mv", name=f"mv{body}_{ct}")
                nc.vector.bn_aggr(out=mv, in_=st)
                m1 = gnp.tile([128, 2], f32, tag="m1", name=f"m1{body}_{ct}")
                nc.vector.tensor_copy(out=m1[:, 0:1], in_=mv[:, 0:1])
                sqm = gnp.tile([128, 1], f32, tag="sqm", name=f"sqm{body}_{ct}")
                nc.vector.tensor_mul(out=sqm, in0=mv[:, 0:1], in1=mv[:, 0:1])
                nc.vector.tensor_add(out=m1[:, 1:2], in0=mv[:, 1:2], in1=sqm)
                gs_ps = stp.tile([8, 2], f32, tag="small", name=f"gs{body}_{ct}")
                nc.tensor.matmul(gs_ps, g_t, m1, start=True, stop=True)
                gsb = gnp.tile([8, 2], f32, tag="gsb", name=f"gsb{body}_{ct}")
                nc.scalar.mul(out=gsb, in_=gs_ps, mul=1.0 / GS)
                t8 = gnp.tile([8, 1], f32, tag="t8", name=f"t8{body}_{ct}")
                nc.vector.tensor_mul(out=t8, in0=gsb[:, 0:1], in1=gsb[:, 0:1])
                vg = gnp.tile([8, 1], f32, tag="vg", name=f"vg{body}_{ct}")
                nc.vector.tensor_sub(out=vg, in0=gsb[:, 1:2], in1=t8)
                nc.scalar.activation(out=vg, in_=vg, func=AF.Sqrt,
                                     bias=eps_t[:8, :], scale=1.0)
                st2 = gnp.tile([8, 2], f32, tag="st2", name=f"st2{body}_{ct}")
                nc.vector.tensor_copy(out=st2[:, 0:1], in_=gsb[:, 0:1])
                nc.vector.reciprocal(out=st2[:, 1:2], in_=vg)
                bc_ps = stp.tile([128, 2], f32, tag="small", name=f"bc{body}_{ct}")
                nc.tensor.matmul(bc_ps, gt_t, st2, start=True, stop=True)
                a_sb = gnp.tile([128, 1], f32, tag="a_sb", name=f"a{body}_{ct}")
                nc.vector.tensor_mul(out=a_sb, in0=bc_ps[:, 1:2],
                                     in1=gns[:, ct:ct + 1])
                t1 = gnp.tile([128, 1], f32, tag="t1", name=f"t1{body}_{ct}")
                nc.vector.tensor_mul(out=t1, in0=bc_ps[:, 0:1], in1=a_sb)
                b_sb = gnp.tile([128, 1], f32, tag="b_sb", name=f"bb{body}_{ct}")
                nc.vector.tensor_sub(out=b_sb, in0=gnb[:, ct:ct + 1], in1=t1)
                ht = big.tile([128, N], f32r, tag=f"h{ct}", name=f"h{body}_{ct}",
                              bufs=h_bufs)
                if ct % 2 == 0 and not opts.get("h_act_all"):
                    nc.vector.tensor_scalar(out=ht, in0=x_t[ct], scalar1=a_sb,
                                            scalar2=b_sb, op0=ALU.mult,
                                            op1=ALU.add)
                else:
                    nc.scalar.activation(out=ht, in_=x_t[ct], func=AF.Identity,
                                         bias=b_sb, scale=a_sb)
                h_t.append(ht)
            st8[body] = {"x_t": x_t, "h_t": h_t}

        def qkv_stage(body):
            h_t = st8[body]["h_t"]
            u_t = [big.tile([128, N], f32r, tag=f"u{ct}", name=f"u{body}_{ct}")
                   for ct in range(NCT)]
            for o in range(NCT):
                for nch in range(NCHK):
                    sl = slice(nch * 512, (nch + 1) * 512)
                    ps = mp.tile([128, 512], f32, tag="mm",
                                 name=f"u{body}_{o}_{nch}")
                    for kt in range(NCT):
                        nc.tensor.matmul(ps, wat[kt][:, o * 128:(o + 1) * 128],
                                         h_t[kt][:, sl],
                                         start=(kt == 0), stop=(kt == NCT - 1))
                    if opts.get("u_act_split") and o % 2 == 1:
                        nc.scalar.copy(out=u_t[o][:, sl], in_=ps)
                    else:
                        nc.vector.tensor_copy(out=u_t[o][:, sl], in_=ps)
            vT_t = [big.tile([128, C], f32r, tag=f"vT{nt}", name=f"vT{body}_{nt}")
                    for nt in range(NNT)]
            for nt in range(NNT):
                ps = mp.tile([128, 512], f32, tag="mm", name=f"v{body}_{nt}")
                for kt in range(NCT):
                    nc.tensor.matmul(ps, h_t[kt][:, nt * 128:(nt + 1) * 128],
                                     wvt[kt],
                                     start=(kt == 0), stop=(kt == NCT - 1))
                if opts.get("vt_act", 1):
                    nc.scalar.copy(out=vT_t[nt], in_=ps)
                else:
                    nc.vector.tensor_copy(out=vT_t[nt], in_=ps)
            st8[body].update(u_t=u_t, vT_t=vT_t)

        def sc_stage(body):
            h_t, u_t = st8[body]["h_t"], st8[body]["u_t"]
            pT_t = [big.tile([128, N], f32r, tag=f"pT{mt}", name=f"pT{body}_{mt}")
                    for mt in range(NNT)]
            for mt in range(NNT):
                for nch in range(NCHK):
                    sl = slice(nch * 512, (nch + 1) * 512)
                    ps = mp.tile([128, 512], f32, tag="mm",
                                 name=f"s{body}_{mt}_{nch}")
                    for kt in range(NCT):
                        nc.tensor.matmul(ps, u_t[kt][:, mt * 128:(mt + 1) * 128],
                                         h_t[kt][:, sl],
                                         start=(kt == 0), stop=(kt == NCT - 1))
                    nc.scalar.activation(out=pT_t[mt][:, sl], in_=ps,
                                         func=AF.Exp, scale=SCALE)
            st8[body]["pT_t"] = pT_t

        def sum_stage(body):
            pT_t = st8[body]["pT_t"]
            rb_sb = []
            for nch in range(NCHK):
                sl = slice(nch * 512, (nch + 1) * 512)
                sum_ps = sump.tile([1, 512], f32, tag="small",
                                   name=f"sm{body}_{nch}")
                for mt in range(NNT):
                    nc.tensor.matmul(sum_ps, ones_col, pT_t[mt][:, sl],
                                     start=(mt == 0), stop=(mt == NNT - 1))
                rc = gnp.tile([1, 512], f32r, tag="rc", name=f"rc{body}_{nch}")
                with nc.allow_low_precision(reason="f32r feed for bcast matmul"):
                    nc.vector.reciprocal(out=rc, in_=sum_ps)
                rb_ps = rbp.tile([128, 512], f32, tag="small",
                                 name=f"rbp{body}_{nch}")
                nc.tensor.matmul(rb_ps, ones_row, rc, start=True, stop=True)
                rb = gnp.tile([128, 512], f32, tag="rb_sb", name=f"rb{body}_{nch}")
                nc.vector.tensor_copy(out=rb, in_=rb_ps)
                rb_sb.append(rb)
            st8[body]["rb_sb"] = rb_sb

        def pv_stage(body):
            vT_t, pT_t, rb_sb = (st8[body][k] for k in ("vT_t", "pT_t", "rb_sb"))
            out_t = [big.tile([128, N], f32r, tag=f"o{ct}", name=f"o{body}_{ct}")
                     for ct in range(NCT)]
            for ct in range(NCT):
                for nch in range(NCHK):
                    sl = slice(nch * 512, (nch + 1) * 512)
                    ps = mp.tile([128, 512], f32, tag="mm",
                                 name=f"pv{body}_{ct}_{nch}")
                    for mt in range(NNT):
                        nc.tensor.matmul(ps,
                                         vT_t[mt][:, ct * 128:(ct + 1) * 128],
                                         pT_t[mt][:, sl],
                                         start=(mt == 0), stop=(mt == NNT - 1))
                    nc.vector.tensor_mul(out=out_t[ct][:, sl], in0=ps,
                                         in1=rb_sb[nch])
            st8[body]["out_t"] = out_t

        def proj_stage(body):
            b = body % BPC
            out_t, x_t = st8[body]["out_t"], st8[body]["x_t"]
            for ot in range(NCT):
                for nch in range(NCHK):
                    sl = slice(nch * 512, (nch + 1) * 512)
                    ps = mp.tile([128, 512], f32, tag="mm",
                                 name=f"pj{body}_{ot}_{nch}")
                    for ct in range(NCT):
                        nc.tensor.matmul(ps, wpt[ct][:, ot * 128:(ot + 1) * 128],
                                         out_t[ct][:, sl],
                                         start=(ct == 0), stop=(ct == NCT - 1))
                    fc = gnp.tile([128, 512], f32, tag="fin",
                                  bufs=opts.get("fin_bufs", 6),
                                  name=f"fin{body}_{ot}_{nch}")
                    nc.scalar.activation(out=fc, in_=ps, func=AF.Identity,
                                         bias=pbe[:, ot:ot + 1], scale=1.0)
                    nc.vector.tensor_add(out=fc, in0=fc, in1=x_t[ot][:, sl])
                    nc.sync.dma_start(
                        out=y_d[b, ot * 128:(ot + 1) * 128, sl], in_=fc)
            st8[body].clear()

        ilv = opts.get("interleave", 1)
        if ilv == 2:
            gn_stage(0); qkv_stage(0)
            for k in range(n_bodies):
                if k + 1 < n_bodies:
                    gn_stage(k + 1)
                sc_stage(k); sum_stage(k); pv_stage(k)
                if k + 1 < n_bodies:
                    qkv_stage(k + 1)
                proj_stage(k)
        elif ilv:
            gn_stage(0); qkv_stage(0); sc_stage(0)
            for k in range(n_bodies):
                if k + 1 < n_bodies:
                    gn_stage(k + 1)
                sum_stage(k); pv_stage(k)
                if k + 1 < n_bodies:
                    qkv_stage(k + 1)
                proj_stage(k)
                if k + 1 < n_bodies:
                    sc_stage(k + 1)
        else:
            for k in range(n_bodies):
                gn_stage(k); qkv_stage(k); sc_stage(k)
                sum_stage(k); pv_stage(k); proj_stage(k)


def build(n_bodies=BPC, **opts):
    nc = bacc.Bacc("TRN2")
    _emit(nc, n_bodies, opts)
    nc.compile()
    return nc


_cached = {}


def get_nc(n_bodies=BPC, **opts):
    key = (n_bodies, tuple(sorted(opts.items())))
    if key not in _cached:
        _cached[key] = build(n_bodies, **opts)
    return _cached[key]


def make_in_maps(x, gn_scale, gn_bias, qkv_w, qkv_b, proj_w, proj_b):
    x = np.ascontiguousarray(np.asarray(x, np.float32).reshape(B, C, N))
    gn_scale = np.asarray(gn_scale, np.float32)
    gn_bias = np.asarray(gn_bias, np.float32)
    qkv_w = np.asarray(qkv_w, np.float32)
    qkv_b = np.asarray(qkv_b, np.float32)
    proj_w = np.asarray(proj_w, np.float32)
    proj_b = np.asarray(proj_b, np.float32)

    assert np.abs(qkv_b[:2 * C]).max() == 0.0, "q/k biases assumed zero"
    wq = qkv_w[0:C].astype(np.float64)        # [C, C] rows o, cols c
    wk = qkv_w[C:2 * C].astype(np.float64)
    A = wq.T @ wk                             # [C(c'), C(c)]; S = h^T A h
    waT = np.ascontiguousarray(A.T.astype(np.float32))   # lhsT layout [c, c']
    wvT = np.ascontiguousarray(qkv_w[2 * C:].T)          # [C, C]
    wpT = np.ascontiguousarray(proj_w.T)                 # [C, C]
    gns = np.ascontiguousarray(gn_scale.reshape(NCT, 128).T)
    gnb = np.ascontiguousarray(gn_bias.reshape(NCT, 128).T)
    pbe_vec = proj_w @ qkv_b[2 * C:] + proj_b                  # fold v-bias
    pbe = np.ascontiguousarray(pbe_vec.astype(np.float32).reshape(NCT, 128).T)
    gsel = np.zeros((128, 8), np.float32)
    gsel[np.arange(128), np.arange(128) // GS] = 1.0
    gselT = np.ascontiguousarray(gsel.T)

    shared = {"wa": waT, "wv": wvT, "wp": wpT, "gns": gns,
              "gnb": gnb, "pbe": pbe, "gsel": gsel, "gselT": gselT}
    return [{"x": np.ascontiguousarray(x[BPC * i:BPC * (i + 1)]), **shared}
            for i in range(NCORES)]


def kernel(x, gn_scale, gn_bias, qkv_w, qkv_b, proj_w, proj_b):
    in_maps = make_in_maps(x, gn_scale, gn_bias, qkv_w, qkv_b, proj_w, proj_b)
    nc = get_nc()
    res = run_bass_kernel_spmd(nc, in_maps, list(range(NCORES)))
    y = np.concatenate([res.results[i]["y"] for i in range(NCORES)], axis=0)
    return np.ascontiguousarray(y.reshape(B, C, H, W).astype(np.float32))
